# revision 1
# baseline (speedup 1.0000x reference)
"""ChebNet (K=3, 3 layers) GNN on 8 Trainium2 NeuronCores.

Math: per layer, out = h@(W0-W2) + L(h@W1 + 2*L(h@W2)) + b, where
L(v) = -dis * S(dis * v), S = unweighted scatter-add over edges, and
dis = rsqrt(clamp(outdeg,1)) masked by outdeg>0.  The per-edge weight
w = dis[src]*dis[dst] folds into two dense per-node row scalings.

Sharding: nodes split into 8 contiguous ranges (one per core, padded to
49*128 rows).  Each core owns the edges whose dst falls in its range and
computes output rows for its range only.  Before each graph op (lap) the
dis-scaled operand is AllGathered so every core can gather arbitrary src
rows with dma_gather.  dma_gather indices are int16, so the 50176-row
gathered tensor is addressed through two windows (rows [0,32768) and
[32768,50176)); each dst-tile's edges are split lo/hi by src window and
the two partial scatter sums merge for free in PSUM accumulation.

Scatter-add is done on the tensor engine: for each 128-edge chunk, a
[128e x 128dst] one-hot matrix (built on the vector engine by comparing
an iota row against the per-edge local dst) is matmul'd against the
gathered [128e x 64f] rows, accumulating [128dst x 64f] in PSUM.
"""

import sys

sys.path.insert(0, "/opt/trn_rl_repo")

import numpy as np
from contextlib import ExitStack

_REAL = dict(N=50000, E=800000, NCORES=8, LO=32768, F0=128, F1=64, F2=16)


# ---------------------------------------------------------------- host prep
def _derive(cfg):
    c = dict(cfg)
    c["NPC"] = c["N"] // c["NCORES"]
    c["NT"] = -(-c["NPC"] // 128)
    c["NPAD"] = c["NT"] * 128
    c["NG"] = c["NCORES"] * c["NPAD"]
    c["FW"] = 64  # lap working width (256B gather elements)
    assert c["LO"] <= 32768 and c["NG"] - c["LO"] <= 32768
    assert c["N"] % c["NCORES"] == 0
    return c


def _prep(edge_index, c):
    N, E, NCORES, LO = c["N"], c["E"], c["NCORES"], c["LO"]
    NPC, NT, NPAD = c["NPC"], c["NT"], c["NPAD"]

    src = np.asarray(edge_index[0], dtype=np.int64)
    dst = np.asarray(edge_index[1], dtype=np.int64)
    assert src.shape == (E,) and dst.shape == (E,)
    psrc = (src // NPC) * NPAD + (src % NPC)  # padded global row of src

    cd = dst // NPC
    ld = dst - cd * NPC
    td = ld >> 7
    dl = ld & 127
    hi = (psrc >= LO).astype(np.int64)

    # ---- lap tables: edges grouped by (core, dst-tile, window), src-sorted
    counts = np.zeros((NCORES, NT, 2), np.int64)
    np.add.at(counts, (cd, td, hi), 1)
    Klo = np.maximum(1, -(-counts[:, :, 0].max(0) // 128))
    Khi = np.maximum(1, -(-counts[:, :, 1].max(0) // 128))
    LOFF = np.concatenate([[0], np.cumsum(Klo)]).astype(np.int64)
    HOFF = np.concatenate([[0], np.cumsum(Khi)]).astype(np.int64)
    TLO, THI = int(LOFF[-1]), int(HOFF[-1])

    order = np.lexsort((psrc, hi, td, cd))
    cd_s, td_s, hi_s = cd[order], td[order], hi[order]
    dl_s, psrc_s = dl[order], psrc[order]
    grp = (cd_s * NT + td_s) * 2 + hi_s
    gc = np.bincount(grp, minlength=NCORES * NT * 2)
    gstart = np.concatenate([[0], np.cumsum(gc)])[:-1]
    rank = np.arange(E) - gstart[grp]

    gidx_lo = np.zeros((NCORES, TLO * 128), np.int16)
    gidx_hi = np.zeros((NCORES, THI * 128), np.int16)
    dloc_lo = np.full((NCORES, 128, TLO), -1.0, np.float32)
    dloc_hi = np.full((NCORES, 128, THI), -1.0, np.float32)
    for cc in range(NCORES):
        for h, (gidx, dloc, OFF, base) in enumerate(
            [(gidx_lo, dloc_lo, LOFF, 0), (gidx_hi, dloc_hi, HOFF, LO)]
        ):
            m = (cd_s == cc) & (hi_s == h)
            slot = OFF[td_s[m]] + rank[m] // 128
            part = rank[m] & 127
            gidx[cc, slot * 128 + part] = (psrc_s[m] - base).astype(np.int16)
            dloc[cc, part, slot] = dl_s[m].astype(np.float32)

    # ---- degree tables: edges grouped by (core, src-tile)
    cs = src // NPC
    ls = src - cs * NPC
    ts_ = ls >> 7
    sl = ls & 127
    dcounts = np.zeros((NCORES, NT), np.int64)
    np.add.at(dcounts, (cs, ts_), 1)
    Kd = np.maximum(1, -(-dcounts.max(0) // 128))
    SOFF = np.concatenate([[0], np.cumsum(Kd)]).astype(np.int64)
    TD = int(SOFF[-1])

    order2 = np.lexsort((ts_, cs))
    cs_s, tt_s, sl_s = cs[order2], ts_[order2], sl[order2]
    grp2 = cs_s * NT + tt_s
    gc2 = np.bincount(grp2, minlength=NCORES * NT)
    gstart2 = np.concatenate([[0], np.cumsum(gc2)])[:-1]
    rank2 = np.arange(E) - gstart2[grp2]
    sloc = np.full((NCORES, 128, TD), -1.0, np.float32)
    for cc in range(NCORES):
        m = cs_s == cc
        slot = SOFF[tt_s[m]] + rank2[m] // 128
        sloc[cc, rank2[m] & 127, slot] = sl_s[m].astype(np.float32)

    def wrap(a):  # int16 [M*128] -> [128, M*8], idx j at [j%16, j//16], x8 replicated
        return np.tile(a.reshape(-1, 16).T, (8, 1)).copy()

    return dict(
        Klo=Klo, Khi=Khi, Kd=Kd, LOFF=LOFF, HOFF=HOFF, SOFF=SOFF,
        TLO=TLO, THI=THI, TD=TD,
        gidx_lo=[wrap(gidx_lo[cc]) for cc in range(NCORES)],
        gidx_hi=[wrap(gidx_hi[cc]) for cc in range(NCORES)],
        dloc_lo=dloc_lo, dloc_hi=dloc_hi, sloc=sloc,
    )


# ---------------------------------------------------------------- device build
def _build(c, pp, Fins, use_bias, stages=99):
    import concourse.bacc as bacc
    import concourse.tile as tile
    from concourse import mybir

    f32, i16 = mybir.dt.float32, mybir.dt.int16
    AOT = mybir.AluOpType
    NT, NPAD, NG, LO, FW = c["NT"], c["NPAD"], c["NG"], c["LO"], c["FW"]
    NCORES, F0, F2 = c["NCORES"], c["F0"], c["F2"]
    TLO, THI, TD = pp["TLO"], pp["THI"], pp["TD"]
    Klo, Khi, Kd = pp["Klo"], pp["Khi"], pp["Kd"]
    LOFF, HOFF, SOFF = pp["LOFF"], pp["HOFF"], pp["SOFF"]
    GRP = 4
    groups = [list(range(g * GRP, min((g + 1) * GRP, NT))) for g in range(-(-NT // GRP))]

    nc = bacc.Bacc(num_devices=NCORES)

    xin = nc.dram_tensor("x", [NPAD, F0], f32, kind="ExternalInput")
    gl_d = nc.dram_tensor("gidx_lo", [128, TLO * 8], i16, kind="ExternalInput")
    gh_d = nc.dram_tensor("gidx_hi", [128, THI * 8], i16, kind="ExternalInput")
    dl_d = nc.dram_tensor("dloc_lo", [128, TLO], f32, kind="ExternalInput")
    dh_d = nc.dram_tensor("dloc_hi", [128, THI], f32, kind="ExternalInput")
    sl_d = nc.dram_tensor("sloc", [128, TD], f32, kind="ExternalInput")
    iota_d = nc.dram_tensor("iota", [128, 128], f32, kind="ExternalInput")
    id_d = nc.dram_tensor("ident", [128, 128], f32, kind="ExternalInput")
    W_d = {}
    for l in range(3):
        for nm in ("wa", "wb", "wc"):
            W_d[nm, l] = nc.dram_tensor(f"{nm}{l}", [Fins[l], FW], f32, kind="ExternalInput")
        if use_bias[l]:
            W_d["br", l] = nc.dram_tensor(f"br{l}", [128, FW], f32, kind="ExternalInput")
    y_d = nc.dram_tensor("y", [NPAD, F2], f32, kind="ExternalOutput")

    ag_in = [nc.dram_tensor(f"agin{i}", [NPAD, FW], f32) for i in range(6)]
    ag_out = [nc.dram_tensor(f"agout{i}", [NG, FW], f32, addr_space="Shared") for i in range(6)]

    xv = xin.rearrange("(t p) f -> p t f", p=128)
    yv = y_d.rearrange("(t p) f -> p t f", p=128)

    with tile.TileContext(nc) as tc, ExitStack() as ctx:
        cst = ctx.enter_context(tc.tile_pool(name="cst", bufs=1))
        big = ctx.enter_context(tc.tile_pool(name="big", bufs=1))
        gp = ctx.enter_context(tc.tile_pool(name="gp", bufs=2))
        ohp = ctx.enter_context(tc.tile_pool(name="ohp", bufs=6))
        smp = ctx.enter_context(tc.tile_pool(name="smp", bufs=6))
        slb = ctx.enter_context(tc.tile_pool(name="slb", bufs=2))
        psA = ctx.enter_context(tc.tile_pool(name="psA", bufs=2, space="PSUM"))
        psT = ctx.enter_context(tc.tile_pool(name="psT", bufs=2, space="PSUM"))
        psD = ctx.enter_context(tc.tile_pool(name="psD", bufs=2, space="PSUM"))

        # constants
        iota = cst.tile([128, 128], f32)
        nc.sync.dma_start(iota[:], iota_d[:])
        ident = cst.tile([128, 128], f32)
        nc.sync.dma_start(ident[:], id_d[:])
        ones = cst.tile([128, 1], f32)
        nc.vector.memset(ones[:], 1.0)
        gl = cst.tile([128, TLO * 8], i16)
        nc.sync.dma_start(gl[:], gl_d[:])
        gh = cst.tile([128, THI * 8], i16)
        nc.sync.dma_start(gh[:], gh_d[:])
        dlo = cst.tile([128, TLO], f32)
        nc.sync.dma_start(dlo[:], dl_d[:])
        dhi = cst.tile([128, THI], f32)
        nc.sync.dma_start(dhi[:], dh_d[:])
        slo = cst.tile([128, TD], f32)
        nc.sync.dma_start(slo[:], sl_d[:])
        Wt = {}
        for k, d in W_d.items():
            Wt[k] = cst.tile([128, FW], f32, name=f"w_{k[0]}_{k[1]}", tag=f"w_{k[0]}_{k[1]}")
            nc.sync.dma_start(Wt[k][: d.shape[0], :], d[:])
        dis = cst.tile([128, NT], f32)
        negdis = cst.tile([128, NT], f32)
        n2dis2 = cst.tile([128, NT], f32)

        # ---------------- degree -> dis tables
        for t in range(NT):
            acc = psA.tile([128, FW], f32, tag="acc")
            for k in range(int(Kd[t])):
                s = int(SOFF[t]) + k
                oh = ohp.tile([128, 128], f32, tag="oh")
                nc.vector.tensor_scalar(oh[:], iota[:], slo[:, s : s + 1], None, AOT.is_equal)
                nc.tensor.matmul(
                    acc[:, :1], oh[:], ones[:],
                    start=(k == 0), stop=(k == int(Kd[t]) - 1),
                )
            m = smp.tile([128, 1], f32, tag="m")
            nc.vector.tensor_scalar_max(m[:], acc[:, :1], 1.0)
            sq = smp.tile([128, 1], f32, tag="sq")
            nc.scalar.sqrt(sq[:], m[:])
            r = smp.tile([128, 1], f32, tag="r")
            nc.vector.reciprocal(r[:], sq[:])
            msk = smp.tile([128, 1], f32, tag="msk")
            nc.vector.tensor_scalar(msk[:], acc[:, :1], 0.0, None, AOT.is_gt)
            nc.vector.tensor_mul(dis[:, t : t + 1], r[:], msk[:])
            nc.vector.tensor_scalar_mul(negdis[:, t : t + 1], dis[:, t : t + 1], -1.0)
            d2 = smp.tile([128, 1], f32, tag="d2")
            nc.vector.tensor_mul(d2[:], dis[:, t : t + 1], dis[:, t : t + 1])
            nc.vector.tensor_scalar_mul(n2dis2[:, t : t + 1], d2[:], -2.0)

        # ---------------- lap helper
        import os as _os
        agmode = _os.environ.get("KAG", "cc")
        lapmode = _os.environ.get("KLAP", "full")
        dummy = cst.tile([128, FW], f32, name="dummy", tag="dummy")
        nc.vector.memset(dummy[:], 0.0)

        def lap(agi, epi):
            """Gather rows of ag_out[agi] per edge, scatter-add per dst tile,
            call epi(t, acc_psum) with the [128,FW] PSUM partial sums."""
            if lapmode == "skip":
                for t in range(NT):
                    acc = psA.tile([128, FW], f32, tag="acc")
                    oh = ohp.tile([128, 128], f32, tag="oh")
                    nc.vector.tensor_scalar(oh[:], iota[:], dlo[:, 0:1], None, AOT.is_equal)
                    nc.tensor.matmul(acc[:], oh[:], dummy[:], start=True, stop=True)
                    epi(t, acc)
                return
            src_lo = ag_out[agi][0:LO, :]
            src_hi = ag_out[agi][LO:NG, :]
            for tl in groups:
                a_lo, b_lo = int(LOFF[tl[0]]), int(LOFF[tl[-1] + 1])
                a_hi, b_hi = int(HOFF[tl[0]]), int(HOFF[tl[-1] + 1])
                nlo, nhi = b_lo - a_lo, b_hi - a_hi
                CAP = 8  # max 128-chunks (1024 idx) per dma_gather call
                glo = gp.tile([128, nlo, FW], f32, tag="glo")
                for o in range(0, nlo, CAP):
                    n = min(CAP, nlo - o)
                    nc.gpsimd.dma_gather(
                        glo[:, o : o + n, :], src_lo,
                        gl[:, (a_lo + o) * 8 : (a_lo + o + n) * 8],
                        num_idxs=n * 128, num_idxs_reg=n * 128, elem_size=FW,
                    )
                ghi_t = gp.tile([128, nhi, FW], f32, tag="ghi")
                for o in range(0, nhi, CAP):
                    n = min(CAP, nhi - o)
                    nc.gpsimd.dma_gather(
                        ghi_t[:, o : o + n, :], src_hi,
                        gh[:, (a_hi + o) * 8 : (a_hi + o + n) * 8],
                        num_idxs=n * 128, num_idxs_reg=n * 128, elem_size=FW,
                    )
                if lapmode == "gatheronly":
                    for t in tl:
                        acc = psA.tile([128, FW], f32, tag="acc")
                        oh = ohp.tile([128, 128], f32, tag="oh")
                        nc.vector.tensor_scalar(oh[:], iota[:], dlo[:, 0:1], None, AOT.is_equal)
                        nc.tensor.matmul(acc[:], oh[:], dummy[:], start=True, stop=True)
                        epi(t, acc)
                    continue
                for t in tl:
                    acc = psA.tile([128, FW], f32, tag="acc")
                    ntot = int(Klo[t]) + int(Khi[t])
                    i = 0
                    for k in range(int(Klo[t])):
                        s = int(LOFF[t]) + k
                        oh = ohp.tile([128, 128], f32, tag="oh")
                        nc.vector.tensor_scalar(
                            oh[:], iota[:], dlo[:, s : s + 1], None, AOT.is_equal
                        )
                        nc.tensor.matmul(
                            acc[:], oh[:], glo[:, s - a_lo, :],
                            start=(i == 0), stop=(i == ntot - 1),
                        )
                        i += 1
                    for k in range(int(Khi[t])):
                        s = int(HOFF[t]) + k
                        oh = ohp.tile([128, 128], f32, tag="oh")
                        nc.vector.tensor_scalar(
                            oh[:], iota[:], dhi[:, s : s + 1], None, AOT.is_equal
                        )
                        nc.tensor.matmul(
                            acc[:], oh[:], ghi_t[:, s - a_hi, :],
                            start=(i == 0), stop=(i == ntot - 1),
                        )
                        i += 1
                    epi(t, acc)

        # ---------------- layers
        h_prev = None
        nlayers = 3 if stages >= 99 else max(0, min(3, stages - 1))
        do_lap = stages >= 3 or stages >= 99
        for l in range(nlayers):
            Fin = Fins[l]
            As = big.tile([128, NT, FW], f32, tag="As")
            Cs1 = big.tile([128, NT, FW], f32, tag="Cs1")
            Oa = big.tile([128, NT, FW], f32, tag="Oa")
            for tl in groups:
                w = len(tl) * 128
                hT = slb.tile([128, GRP * 128], f32, tag="hT")
                hsT = slb.tile([128, GRP * 128], f32, tag="hsT")
                for u, t in enumerate(tl):
                    if l == 0:
                        ht = smp.tile([128, F0], f32, tag="xt")
                        nc.sync.dma_start(ht[:], xv[:, t, :])
                        ht_ap = ht[:]
                    else:
                        ht_ap = h_prev[:, t, :]
                    ps = psT.tile([128, 128], f32, tag="pt")
                    nc.tensor.transpose(ps[:Fin, :], ht_ap, ident[:])
                    nc.scalar.copy(hT[:Fin, u * 128 : (u + 1) * 128], ps[:Fin, :])
                    hs = smp.tile([128, Fin], f32, tag="hs")
                    nc.vector.tensor_scalar_mul(hs[:], ht_ap, dis[:, t : t + 1])
                    ps2 = psT.tile([128, 128], f32, tag="pt")
                    nc.tensor.transpose(ps2[:Fin, :], hs[:], ident[:])
                    nc.scalar.copy(hsT[:Fin, u * 128 : (u + 1) * 128], ps2[:Fin, :])
                for dstbuf, wkey, srcT in (
                    (As, ("wc", l), hsT),
                    (Cs1, ("wb", l), hsT),
                    (Oa, ("wa", l), hT),
                ):
                    pd = psD.tile([64, GRP * 128], f32, tag="pd")
                    nc.tensor.matmul(pd[:, :w], Wt[wkey][:Fin, :], srcT[:Fin, :w])
                    pT = slb.tile([64, GRP * 128], f32, tag="pT")
                    nc.scalar.copy(pT[:, :w], pd[:, :w])
                    for u, t in enumerate(tl):
                        pb = psT.tile([128, 128], f32, tag="pt")
                        nc.tensor.transpose(
                            pb[:, :FW], pT[:FW, u * 128 : (u + 1) * 128], ident[:FW, :FW]
                        )
                        nc.scalar.copy(dstbuf[:, t, :], pb[:, :FW])

            if not do_lap:
                h_prev = As
                continue
            agA = 2 * l
            nc.sync.dma_start(ag_in[agA].rearrange("(t p) f -> p t f", p=128), As[:])
            if agmode == "cc":
                nc.gpsimd.collective_compute(
                    "AllGather", mybir.AluOpType.bypass,
                    replica_groups=[list(range(NCORES))],
                    ins=[ag_in[agA][:, :]], outs=[ag_out[agA][:, :]],
                )
            else:
                nc.sync.dma_start(ag_out[agA][0:NPAD, :], ag_in[agA][:, :])

            Cs = big.tile([128, NT, FW], f32, tag="Cs")

            def epi1(t, acc):
                tmp = smp.tile([128, FW], f32, tag="t1")
                nc.vector.tensor_scalar_mul(tmp[:], acc[:], n2dis2[:, t : t + 1])
                nc.vector.tensor_add(Cs[:, t, :], Cs1[:, t, :], tmp[:])

            lap(agA, epi1)

            agC = 2 * l + 1
            nc.sync.dma_start(ag_in[agC].rearrange("(t p) f -> p t f", p=128), Cs[:])
            if agmode == "cc":
                nc.gpsimd.collective_compute(
                    "AllGather", mybir.AluOpType.bypass,
                    replica_groups=[list(range(NCORES))],
                    ins=[ag_in[agC][:, :]], outs=[ag_out[agC][:, :]],
                )
            else:
                nc.sync.dma_start(ag_out[agC][0:NPAD, :], ag_in[agC][:, :])

            hn = big.tile([128, NT, FW], f32, tag=f"h{l % 2}")

            def epi2(t, acc):
                tmp = smp.tile([128, FW], f32, tag="t1")
                nc.vector.tensor_scalar_mul(tmp[:], acc[:], negdis[:, t : t + 1])
                if use_bias[l]:
                    tmp2 = smp.tile([128, FW], f32, tag="t2")
                    nc.vector.tensor_add(tmp2[:], tmp[:], Oa[:, t, :])
                    pre = smp.tile([128, FW], f32, tag="t3")
                    nc.vector.tensor_add(pre[:], tmp2[:], Wt["br", l][:, :])
                else:
                    pre = smp.tile([128, FW], f32, tag="t2")
                    nc.vector.tensor_add(pre[:], tmp[:], Oa[:, t, :])
                if l < 2:
                    nc.vector.tensor_scalar_max(hn[:, t, :], pre[:], 0.0)
                else:
                    nc.vector.tensor_copy(hn[:, t, :], pre[:])

            lap(agC, epi2)
            h_prev = hn

        if h_prev is not None:
            nc.sync.dma_start(yv[:], h_prev[:, :, :F2])
        else:
            zt = big.tile([128, NT, FW], f32, tag="zt")
            nc.vector.memset(zt[:], 0.0)
            nc.sync.dma_start(yv[:], zt[:, :, :F2])

    nc.compile()
    return nc


# ---------------------------------------------------------------- entry
def _run(x, edge_index, Ws, bs, cfg=None, trace=False):
    from concourse.bass_utils import run_bass_kernel_spmd

    c = _derive(cfg or _REAL)
    N, NCORES, NPC, NPAD = c["N"], c["NCORES"], c["NPC"], c["NPAD"]
    F0, F2, FW = c["F0"], c["F2"], c["FW"]

    x = np.ascontiguousarray(np.asarray(x, dtype=np.float32))
    pp = _prep(edge_index, c)

    Fins = [F0, c["F1"], c["F1"]]
    use_bias = [bool(np.any(b)) for b in bs]
    nc = _build(c, pp, Fins, use_bias, stages=int(__import__('os').environ.get('KSTAGES', '99')))

    iota = np.tile(np.arange(128, dtype=np.float32), (128, 1))
    ident = np.eye(128, dtype=np.float32)

    def padW(w, fin):
        out = np.zeros((fin, FW), np.float32)
        out[: w.shape[0], : w.shape[1]] = w
        return out

    base = {"iota": iota, "ident": ident}
    for l in range(3):
        W = np.asarray(Ws[l], dtype=np.float32)
        base[f"wa{l}"] = padW(W[0] - W[2], Fins[l])
        base[f"wb{l}"] = padW(W[1], Fins[l])
        base[f"wc{l}"] = padW(W[2], Fins[l])
        if use_bias[l]:
            br = np.zeros((128, FW), np.float32)
            br[:, : bs[l].shape[0]] = np.asarray(bs[l], np.float32)
            base[f"br{l}"] = br

    in_maps = []
    for cc in range(NCORES):
        xl = np.zeros((NPAD, F0), np.float32)
        xl[:NPC] = x[cc * NPC : (cc + 1) * NPC]
        in_maps.append(
            dict(
                base,
                x=xl,
                gidx_lo=pp["gidx_lo"][cc],
                gidx_hi=pp["gidx_hi"][cc],
                dloc_lo=np.ascontiguousarray(pp["dloc_lo"][cc]),
                dloc_hi=np.ascontiguousarray(pp["dloc_hi"][cc]),
                sloc=np.ascontiguousarray(pp["sloc"][cc]),
            )
        )

    res = run_bass_kernel_spmd(nc, in_maps, core_ids=list(range(NCORES)), trace=trace)
    out = np.concatenate([res.results[cc]["y"][:NPC] for cc in range(NCORES)], axis=0)
    return out[:, :F2], res


def kernel(x, edge_index, W1, b1, Wm, bm, W2, b2):
    out, _ = _run(
        np.asarray(x), np.asarray(edge_index),
        [np.asarray(W1), np.asarray(Wm), np.asarray(W2)],
        [np.asarray(b1), np.asarray(bm), np.asarray(b2)],
    )
    return out



# revision 2
# speedup vs baseline: 1.6312x; 1.6312x over previous
"""ChebNet (K=3, 3 layers) GNN on 8 Trainium2 NeuronCores.

Math: per layer, out = h@(W0-W2) + L(h@W1 + 2*L(h@W2)) + b, where
L(v) = -dis * S(dis * v), S = unweighted scatter-add over edges, and
dis = rsqrt(clamp(outdeg,1)) masked by outdeg>0.  The per-edge weight
w = dis[src]*dis[dst] folds into two dense per-node row scalings.

Sharding: nodes split into 8 contiguous ranges (one per core, padded to
49*128 rows).  Each core owns the edges whose dst falls in its range and
computes output rows for its range only.  Before each graph op (lap) the
dis-scaled operand is AllGathered so every core can gather arbitrary src
rows with dma_gather.  dma_gather indices are int16, so the 50176-row
gathered tensor is addressed through two windows (rows [0,32768) and
[32768,50176)); each dst-tile's edges are split lo/hi by src window and
the two partial scatter sums merge for free in PSUM accumulation.

Scatter-add is done on the tensor engine: for each 128-edge chunk, a
[128e x 128dst] one-hot matrix is matmul'd against the gathered
[128e x 64f] rows, accumulating [128dst x 64f] in PSUM.  The one-hot
matrices depend only on the graph: they are built on the vector engine
once (first lap), stored to DRAM, and streamed back by DMA for the
remaining 5 laps.  dis tables come precomputed from the host.  Gather
calls rotate across 4 SWDGE queues (distinct Q7 core pairs + rings).
"""

import sys

sys.path.insert(0, "/opt/trn_rl_repo")

import numpy as np
from contextlib import ExitStack

_REAL = dict(N=50000, E=800000, NCORES=8, LO=32768, F0=128, F1=64, F2=16)


# ---------------------------------------------------------------- host prep
def _derive(cfg):
    c = dict(cfg)
    c["NPC"] = c["N"] // c["NCORES"]
    c["NT"] = -(-c["NPC"] // 128)
    c["NPAD"] = c["NT"] * 128
    c["NG"] = c["NCORES"] * c["NPAD"]
    c["FW"] = 64  # lap working width (256B gather elements)
    assert c["LO"] <= 32768 and c["NG"] - c["LO"] <= 32768
    assert c["N"] % c["NCORES"] == 0
    return c


def _prep(edge_index, c):
    N, E, NCORES, LO = c["N"], c["E"], c["NCORES"], c["LO"]
    NPC, NT, NPAD = c["NPC"], c["NT"], c["NPAD"]

    src = np.asarray(edge_index[0], dtype=np.int64)
    dst = np.asarray(edge_index[1], dtype=np.int64)
    assert src.shape == (E,) and dst.shape == (E,)
    psrc = (src // NPC) * NPAD + (src % NPC)  # padded global row of src

    cd = dst // NPC
    ld = dst - cd * NPC
    td = ld >> 7
    dl = ld & 127
    hi = (psrc >= LO).astype(np.int64)

    # ---- lap tables: edges grouped by (core, dst-tile, window), src-sorted
    counts = np.zeros((NCORES, NT, 2), np.int64)
    np.add.at(counts, (cd, td, hi), 1)
    Klo = np.maximum(1, -(-counts[:, :, 0].max(0) // 128))
    Khi = np.maximum(1, -(-counts[:, :, 1].max(0) // 128))
    LOFF = np.concatenate([[0], np.cumsum(Klo)]).astype(np.int64)
    HOFF = np.concatenate([[0], np.cumsum(Khi)]).astype(np.int64)
    TLO, THI = int(LOFF[-1]), int(HOFF[-1])

    order = np.lexsort((psrc, hi, td, cd))
    cd_s, td_s, hi_s = cd[order], td[order], hi[order]
    dl_s, psrc_s = dl[order], psrc[order]
    grp = (cd_s * NT + td_s) * 2 + hi_s
    gc = np.bincount(grp, minlength=NCORES * NT * 2)
    gstart = np.concatenate([[0], np.cumsum(gc)])[:-1]
    rank = np.arange(E) - gstart[grp]

    gidx_lo = np.zeros((NCORES, TLO * 128), np.int16)
    gidx_hi = np.zeros((NCORES, THI * 128), np.int16)
    dloc_lo = np.full((NCORES, 128, TLO), -1.0, np.float32)
    dloc_hi = np.full((NCORES, 128, THI), -1.0, np.float32)
    for cc in range(NCORES):
        for h, (gidx, dloc, OFF, base) in enumerate(
            [(gidx_lo, dloc_lo, LOFF, 0), (gidx_hi, dloc_hi, HOFF, LO)]
        ):
            m = (cd_s == cc) & (hi_s == h)
            slot = OFF[td_s[m]] + rank[m] // 128
            part = rank[m] & 127
            gidx[cc, slot * 128 + part] = (psrc_s[m] - base).astype(np.int16)
            dloc[cc, part, slot] = dl_s[m].astype(np.float32)

    # ---- dis tables from host-side degrees (replaces on-device degree pass)
    deg = np.bincount(src, minlength=N).astype(np.float64)
    dis_node = np.where(deg > 0, 1.0 / np.sqrt(np.maximum(deg, 1.0)), 0.0)
    dis_t = np.zeros((NCORES, 128, NT), np.float32)
    for cc in range(NCORES):
        dn = np.zeros(NPAD, np.float64)
        dn[:NPC] = dis_node[cc * NPC : (cc + 1) * NPC]
        dis_t[cc] = dn.reshape(NT, 128).T.astype(np.float32)

    def wrap(a):  # int16 [M*128] -> [128, M*8], idx j at [j%16, j//16], x8 replicated
        return np.tile(a.reshape(-1, 16).T, (8, 1)).copy()

    return dict(
        Klo=Klo, Khi=Khi, LOFF=LOFF, HOFF=HOFF, TLO=TLO, THI=THI,
        gidx_lo=[wrap(gidx_lo[cc]) for cc in range(NCORES)],
        gidx_hi=[wrap(gidx_hi[cc]) for cc in range(NCORES)],
        dloc_lo=dloc_lo, dloc_hi=dloc_hi,
        dis=dis_t, negdis=-dis_t, n2dis2=(-2.0 * dis_t * dis_t).astype(np.float32),
    )


# ---------------------------------------------------------------- device build
def _build(c, pp, Fins, use_bias):
    import concourse.bacc as bacc
    import concourse.tile as tile
    from concourse import mybir

    f32, i16 = mybir.dt.float32, mybir.dt.int16
    AOT = mybir.AluOpType
    NT, NPAD, NG, LO, FW = c["NT"], c["NPAD"], c["NG"], c["LO"], c["FW"]
    NCORES, F0, F2 = c["NCORES"], c["F0"], c["F2"]
    TLO, THI = pp["TLO"], pp["THI"]
    Klo, Khi = pp["Klo"], pp["Khi"]
    LOFF, HOFF = pp["LOFF"], pp["HOFF"]
    KMAX = int(max(Klo.max(), Khi.max()))
    NQ = 4  # SWDGE queues to rotate gather calls over
    GRP = 4
    groups = [list(range(g * GRP, min((g + 1) * GRP, NT))) for g in range(-(-NT // GRP))]

    nc = bacc.Bacc(num_devices=NCORES, num_swdge_queues=NQ)

    xin = nc.dram_tensor("x", [NPAD, F0], f32, kind="ExternalInput")
    gl_d = nc.dram_tensor("gidx_lo", [128, TLO * 8], i16, kind="ExternalInput")
    gh_d = nc.dram_tensor("gidx_hi", [128, THI * 8], i16, kind="ExternalInput")
    dl_d = nc.dram_tensor("dloc_lo", [128, TLO], f32, kind="ExternalInput")
    dh_d = nc.dram_tensor("dloc_hi", [128, THI], f32, kind="ExternalInput")
    dis_d = nc.dram_tensor("dis", [128, NT], f32, kind="ExternalInput")
    ndis_d = nc.dram_tensor("negdis", [128, NT], f32, kind="ExternalInput")
    n2d2_d = nc.dram_tensor("n2dis2", [128, NT], f32, kind="ExternalInput")
    iota_d = nc.dram_tensor("iota", [128, 128], f32, kind="ExternalInput")
    id_d = nc.dram_tensor("ident", [128, 128], f32, kind="ExternalInput")
    W_d = {}
    for l in range(3):
        for nm in ("wa", "wb", "wc"):
            W_d[nm, l] = nc.dram_tensor(f"{nm}{l}", [Fins[l], FW], f32, kind="ExternalInput")
        if use_bias[l]:
            W_d["br", l] = nc.dram_tensor(f"br{l}", [128, FW], f32, kind="ExternalInput")
    y_d = nc.dram_tensor("y", [NPAD, F2], f32, kind="ExternalOutput")

    ag_in = [nc.dram_tensor(f"agin{i}", [NPAD, FW], f32) for i in range(6)]
    ag_out = [nc.dram_tensor(f"agout{i}", [NG, FW], f32, addr_space="Shared") for i in range(6)]
    oh_lo_d = nc.dram_tensor("ohlo", [128, TLO * 128], f32)
    oh_hi_d = nc.dram_tensor("ohhi", [128, THI * 128], f32)

    xv = xin.rearrange("(t p) f -> p t f", p=128)
    yv = y_d.rearrange("(t p) f -> p t f", p=128)

    with tile.TileContext(nc) as tc, ExitStack() as ctx:
        cst = ctx.enter_context(tc.tile_pool(name="cst", bufs=1))
        big = ctx.enter_context(tc.tile_pool(name="big", bufs=1))
        gp = ctx.enter_context(tc.tile_pool(name="gp", bufs=2))
        ohp = ctx.enter_context(tc.tile_pool(name="ohp", bufs=2))
        smp = ctx.enter_context(tc.tile_pool(name="smp", bufs=6))
        slb = ctx.enter_context(tc.tile_pool(name="slb", bufs=2))
        psA = ctx.enter_context(tc.tile_pool(name="psA", bufs=2, space="PSUM"))
        psT = ctx.enter_context(tc.tile_pool(name="psT", bufs=2, space="PSUM"))
        psD = ctx.enter_context(tc.tile_pool(name="psD", bufs=2, space="PSUM"))

        # constants
        iota = cst.tile([128, 128], f32)
        nc.sync.dma_start(iota[:], iota_d[:])
        ident = cst.tile([128, 128], f32)
        nc.sync.dma_start(ident[:], id_d[:])
        gl = cst.tile([128, TLO * 8], i16)
        nc.sync.dma_start(gl[:], gl_d[:])
        gh = cst.tile([128, THI * 8], i16)
        nc.sync.dma_start(gh[:], gh_d[:])
        dlo = cst.tile([128, TLO], f32)
        nc.sync.dma_start(dlo[:], dl_d[:])
        dhi = cst.tile([128, THI], f32)
        nc.sync.dma_start(dhi[:], dh_d[:])
        dis = cst.tile([128, NT], f32)
        nc.sync.dma_start(dis[:], dis_d[:])
        negdis = cst.tile([128, NT], f32)
        nc.sync.dma_start(negdis[:], ndis_d[:])
        n2dis2 = cst.tile([128, NT], f32)
        nc.sync.dma_start(n2dis2[:], n2d2_d[:])
        Wt = {}
        for k, d in W_d.items():
            Wt[k] = cst.tile([128, FW], f32, name=f"w_{k[0]}_{k[1]}", tag=f"w_{k[0]}_{k[1]}")
            nc.sync.dma_start(Wt[k][: d.shape[0], :], d[:])

        # ---------------- lap helper
        qctr = [0]

        def lap(agi, epi):
            """Gather rows of ag_out[agi] per edge, scatter-add per dst tile,
            call epi(t, acc_psum) with the [128,FW] PSUM partial sums.
            agi==0 builds the one-hot slabs on DVE and stores them to DRAM;
            agi>0 streams them back instead."""
            build = agi == 0
            src_lo = ag_out[agi][0:LO, :]
            src_hi = ag_out[agi][LO:NG, :]
            for tl in groups:
                a_lo, b_lo = int(LOFF[tl[0]]), int(LOFF[tl[-1] + 1])
                a_hi, b_hi = int(HOFF[tl[0]]), int(HOFF[tl[-1] + 1])
                nlo, nhi = b_lo - a_lo, b_hi - a_hi
                CAP = 8  # max 128-chunks (1024 idx) per dma_gather call
                glo = gp.tile([128, nlo, FW], f32, tag="glo")
                for o in range(0, nlo, CAP):
                    n = min(CAP, nlo - o)
                    nc.gpsimd.dma_gather(
                        glo[:, o : o + n, :], src_lo,
                        gl[:, (a_lo + o) * 8 : (a_lo + o + n) * 8],
                        num_idxs=n * 128, num_idxs_reg=n * 128, elem_size=FW,
                        queue_num=qctr[0] % NQ,
                    )
                    qctr[0] += 1
                ghi_t = gp.tile([128, nhi, FW], f32, tag="ghi")
                for o in range(0, nhi, CAP):
                    n = min(CAP, nhi - o)
                    nc.gpsimd.dma_gather(
                        ghi_t[:, o : o + n, :], src_hi,
                        gh[:, (a_hi + o) * 8 : (a_hi + o + n) * 8],
                        num_idxs=n * 128, num_idxs_reg=n * 128, elem_size=FW,
                        queue_num=qctr[0] % NQ,
                    )
                    qctr[0] += 1
                for t in tl:
                    klo, khi = int(Klo[t]), int(Khi[t])
                    slab_lo = ohp.tile([128, KMAX * 128], f32, tag="slab_lo")
                    slab_hi = ohp.tile([128, KMAX * 128], f32, tag="slab_hi")
                    if build:
                        for k in range(klo):
                            s = int(LOFF[t]) + k
                            nc.vector.tensor_scalar(
                                slab_lo[:, k * 128 : (k + 1) * 128],
                                iota[:], dlo[:, s : s + 1], None, AOT.is_equal,
                            )
                        for k in range(khi):
                            s = int(HOFF[t]) + k
                            nc.vector.tensor_scalar(
                                slab_hi[:, k * 128 : (k + 1) * 128],
                                iota[:], dhi[:, s : s + 1], None, AOT.is_equal,
                            )
                        nc.sync.dma_start(
                            oh_lo_d[:, int(LOFF[t]) * 128 : (int(LOFF[t]) + klo) * 128],
                            slab_lo[:, : klo * 128],
                        )
                        nc.sync.dma_start(
                            oh_hi_d[:, int(HOFF[t]) * 128 : (int(HOFF[t]) + khi) * 128],
                            slab_hi[:, : khi * 128],
                        )
                    else:
                        nc.sync.dma_start(
                            slab_lo[:, : klo * 128],
                            oh_lo_d[:, int(LOFF[t]) * 128 : (int(LOFF[t]) + klo) * 128],
                        )
                        nc.sync.dma_start(
                            slab_hi[:, : khi * 128],
                            oh_hi_d[:, int(HOFF[t]) * 128 : (int(HOFF[t]) + khi) * 128],
                        )
                    acc = psA.tile([128, FW], f32, tag="acc")
                    ntot = klo + khi
                    i = 0
                    for k in range(klo):
                        s = int(LOFF[t]) + k
                        nc.tensor.matmul(
                            acc[:], slab_lo[:, k * 128 : (k + 1) * 128],
                            glo[:, s - a_lo, :],
                            start=(i == 0), stop=(i == ntot - 1),
                        )
                        i += 1
                    for k in range(khi):
                        s = int(HOFF[t]) + k
                        nc.tensor.matmul(
                            acc[:], slab_hi[:, k * 128 : (k + 1) * 128],
                            ghi_t[:, s - a_hi, :],
                            start=(i == 0), stop=(i == ntot - 1),
                        )
                        i += 1
                    epi(t, acc)

        # ---------------- layers
        h_prev = None
        for l in range(3):
            Fin = Fins[l]
            As = big.tile([128, NT, FW], f32, tag="As")
            Cs1 = big.tile([128, NT, FW], f32, tag="Cs1")
            Oa = big.tile([128, NT, FW], f32, tag="Oa")
            for tl in groups:
                w = len(tl) * 128
                hT = slb.tile([128, GRP * 128], f32, tag="hT")
                hsT = slb.tile([128, GRP * 128], f32, tag="hsT")
                for u, t in enumerate(tl):
                    if l == 0:
                        ht = smp.tile([128, F0], f32, tag="xt")
                        nc.sync.dma_start(ht[:], xv[:, t, :])
                        ht_ap = ht[:]
                    else:
                        ht_ap = h_prev[:, t, :]
                    ps = psT.tile([128, 128], f32, tag="pt")
                    nc.tensor.transpose(ps[:Fin, :], ht_ap, ident[:])
                    nc.scalar.copy(hT[:Fin, u * 128 : (u + 1) * 128], ps[:Fin, :])
                    hs = smp.tile([128, Fin], f32, tag="hs")
                    nc.vector.tensor_scalar_mul(hs[:], ht_ap, dis[:, t : t + 1])
                    ps2 = psT.tile([128, 128], f32, tag="pt")
                    nc.tensor.transpose(ps2[:Fin, :], hs[:], ident[:])
                    nc.scalar.copy(hsT[:Fin, u * 128 : (u + 1) * 128], ps2[:Fin, :])
                for dstbuf, wkey, srcT in (
                    (As, ("wc", l), hsT),
                    (Cs1, ("wb", l), hsT),
                    (Oa, ("wa", l), hT),
                ):
                    pd = psD.tile([64, GRP * 128], f32, tag="pd")
                    nc.tensor.matmul(pd[:, :w], Wt[wkey][:Fin, :], srcT[:Fin, :w])
                    pT = slb.tile([64, GRP * 128], f32, tag="pT")
                    nc.scalar.copy(pT[:, :w], pd[:, :w])
                    for u, t in enumerate(tl):
                        pb = psT.tile([128, 128], f32, tag="pt")
                        nc.tensor.transpose(
                            pb[:, :FW], pT[:FW, u * 128 : (u + 1) * 128], ident[:FW, :FW]
                        )
                        nc.scalar.copy(dstbuf[:, t, :], pb[:, :FW])

            agA = 2 * l
            nc.sync.dma_start(ag_in[agA].rearrange("(t p) f -> p t f", p=128), As[:])
            nc.gpsimd.collective_compute(
                "AllGather", mybir.AluOpType.bypass,
                replica_groups=[list(range(NCORES))],
                ins=[ag_in[agA][:, :]], outs=[ag_out[agA][:, :]],
            )

            Cs = big.tile([128, NT, FW], f32, tag="Cs")

            def epi1(t, acc):
                tmp = smp.tile([128, FW], f32, tag="t1")
                nc.vector.tensor_scalar_mul(tmp[:], acc[:], n2dis2[:, t : t + 1])
                nc.vector.tensor_add(Cs[:, t, :], Cs1[:, t, :], tmp[:])

            lap(agA, epi1)

            agC = 2 * l + 1
            nc.sync.dma_start(ag_in[agC].rearrange("(t p) f -> p t f", p=128), Cs[:])
            nc.gpsimd.collective_compute(
                "AllGather", mybir.AluOpType.bypass,
                replica_groups=[list(range(NCORES))],
                ins=[ag_in[agC][:, :]], outs=[ag_out[agC][:, :]],
            )

            hn = big.tile([128, NT, FW], f32, tag=f"h{l % 2}")

            def epi2(t, acc):
                tmp = smp.tile([128, FW], f32, tag="t1")
                nc.vector.tensor_scalar_mul(tmp[:], acc[:], negdis[:, t : t + 1])
                if use_bias[l]:
                    tmp2 = smp.tile([128, FW], f32, tag="t2")
                    nc.vector.tensor_add(tmp2[:], tmp[:], Oa[:, t, :])
                    pre = smp.tile([128, FW], f32, tag="t3")
                    nc.vector.tensor_add(pre[:], tmp2[:], Wt["br", l][:, :])
                else:
                    pre = smp.tile([128, FW], f32, tag="t2")
                    nc.vector.tensor_add(pre[:], tmp[:], Oa[:, t, :])
                if l < 2:
                    nc.vector.tensor_scalar_max(hn[:, t, :], pre[:], 0.0)
                else:
                    nc.vector.tensor_copy(hn[:, t, :], pre[:])

            lap(agC, epi2)
            h_prev = hn

        nc.sync.dma_start(yv[:], h_prev[:, :, :F2])

    nc.compile()
    return nc


# ---------------------------------------------------------------- entry
def _run(x, edge_index, Ws, bs, cfg=None, trace=False):
    from concourse.bass_utils import run_bass_kernel_spmd

    c = _derive(cfg or _REAL)
    N, NCORES, NPC, NPAD = c["N"], c["NCORES"], c["NPC"], c["NPAD"]
    F0, F2, FW = c["F0"], c["F2"], c["FW"]

    x = np.ascontiguousarray(np.asarray(x, dtype=np.float32))
    pp = _prep(edge_index, c)

    Fins = [F0, c["F1"], c["F1"]]
    use_bias = [bool(np.any(b)) for b in bs]
    nc = _build(c, pp, Fins, use_bias)

    iota = np.tile(np.arange(128, dtype=np.float32), (128, 1))
    ident = np.eye(128, dtype=np.float32)

    def padW(w, fin):
        out = np.zeros((fin, FW), np.float32)
        out[: w.shape[0], : w.shape[1]] = w
        return out

    base = {"iota": iota, "ident": ident}
    for l in range(3):
        W = np.asarray(Ws[l], dtype=np.float32)
        base[f"wa{l}"] = padW(W[0] - W[2], Fins[l])
        base[f"wb{l}"] = padW(W[1], Fins[l])
        base[f"wc{l}"] = padW(W[2], Fins[l])
        if use_bias[l]:
            br = np.zeros((128, FW), np.float32)
            br[:, : bs[l].shape[0]] = np.asarray(bs[l], np.float32)
            base[f"br{l}"] = br

    in_maps = []
    for cc in range(NCORES):
        xl = np.zeros((NPAD, F0), np.float32)
        xl[:NPC] = x[cc * NPC : (cc + 1) * NPC]
        in_maps.append(
            dict(
                base,
                x=xl,
                gidx_lo=pp["gidx_lo"][cc],
                gidx_hi=pp["gidx_hi"][cc],
                dloc_lo=np.ascontiguousarray(pp["dloc_lo"][cc]),
                dloc_hi=np.ascontiguousarray(pp["dloc_hi"][cc]),
                dis=np.ascontiguousarray(pp["dis"][cc]),
                negdis=np.ascontiguousarray(pp["negdis"][cc]),
                n2dis2=np.ascontiguousarray(pp["n2dis2"][cc]),
            )
        )

    res = run_bass_kernel_spmd(nc, in_maps, core_ids=list(range(NCORES)), trace=trace)
    out = np.concatenate([res.results[cc]["y"][:NPC] for cc in range(NCORES)], axis=0)
    return out[:, :F2], res


def kernel(x, edge_index, W1, b1, Wm, bm, W2, b2):
    out, _ = _run(
        np.asarray(x), np.asarray(edge_index),
        [np.asarray(W1), np.asarray(Wm), np.asarray(W2)],
        [np.asarray(b1), np.asarray(bm), np.asarray(b2)],
    )
    return out


# revision 6
# speedup vs baseline: 1.8879x; 1.1574x over previous
"""ChebNet (K=3, 3 layers) GNN on 8 Trainium2 NeuronCores.

Math: per layer, out = h@(W0-W2) + L(h@W1 + 2*L(h@W2)) + b, where
L(v) = -dis * S(dis * v), S = unweighted scatter-add over edges, and
dis = rsqrt(clamp(outdeg,1)) masked by outdeg>0.  The per-edge weight
w = dis[src]*dis[dst] folds into two dense per-node row scalings.

Sharding: nodes split into 8 contiguous ranges (one per core, padded to
49*128 rows).  Each core owns the edges whose dst falls in its range and
computes output rows for its range only.  Before each graph op (lap) the
dis-scaled operand is AllGathered so every core can gather arbitrary src
rows with dma_gather.  dma_gather indices are int16, so the 50176-row
gathered tensor is addressed through two windows (rows [0,32768) and
[32768,50176)); each dst-tile's edges are split lo/hi by src window and
the two partial scatter sums merge for free in PSUM accumulation.

Scatter-add is done on the tensor engine: for each 128-edge chunk, a
[128e x 128dst] one-hot matrix is matmul'd against the gathered
[128e x 64f] rows, accumulating [128dst x 64f] in PSUM.  The one-hot
matrices depend only on the graph: they are built on the vector engine
once (first lap), stored to DRAM, and streamed back by DMA for the
remaining 5 laps.  dis tables come precomputed from the host.  Gather
calls rotate across 4 SWDGE queues (distinct Q7 core pairs + rings).
"""

import sys

sys.path.insert(0, "/opt/trn_rl_repo")

import numpy as np
from contextlib import ExitStack

_REAL = dict(N=50000, E=800000, NCORES=8, LO=32768, F0=128, F1=64, F2=16)


# ---------------------------------------------------------------- host prep
def _derive(cfg):
    c = dict(cfg)
    c["NPC"] = c["N"] // c["NCORES"]
    c["NT"] = -(-c["NPC"] // 128)
    c["NPAD"] = c["NT"] * 128
    c["NG"] = c["NCORES"] * c["NPAD"]
    c["FW"] = 64  # lap working width (256B gather elements)
    assert c["LO"] <= 32768 and c["NG"] - c["LO"] <= 32768
    assert c["N"] % c["NCORES"] == 0
    return c


def _prep(edge_index, c):
    N, E, NCORES, LO = c["N"], c["E"], c["NCORES"], c["LO"]
    NPC, NT, NPAD = c["NPC"], c["NT"], c["NPAD"]

    src = np.asarray(edge_index[0], dtype=np.int64)
    dst = np.asarray(edge_index[1], dtype=np.int64)
    assert src.shape == (E,) and dst.shape == (E,)
    psrc = (src // NPC) * NPAD + (src % NPC)  # padded global row of src

    cd = dst // NPC
    ld = dst - cd * NPC
    td = ld >> 7
    dl = ld & 127
    hi = (psrc >= LO).astype(np.int64)

    # ---- lap tables: edges grouped by (core, dst-tile, window), src-sorted
    counts = np.zeros((NCORES, NT, 2), np.int64)
    np.add.at(counts, (cd, td, hi), 1)
    Klo = np.maximum(1, -(-counts[:, :, 0].max(0) // 128))
    Khi = np.maximum(1, -(-counts[:, :, 1].max(0) // 128))
    LOFF = np.concatenate([[0], np.cumsum(Klo)]).astype(np.int64)
    HOFF = np.concatenate([[0], np.cumsum(Khi)]).astype(np.int64)
    TLO, THI = int(LOFF[-1]), int(HOFF[-1])

    order = np.lexsort((psrc, hi, td, cd))
    cd_s, td_s, hi_s = cd[order], td[order], hi[order]
    dl_s, psrc_s = dl[order], psrc[order]
    grp = (cd_s * NT + td_s) * 2 + hi_s
    gc = np.bincount(grp, minlength=NCORES * NT * 2)
    gstart = np.concatenate([[0], np.cumsum(gc)])[:-1]
    rank = np.arange(E) - gstart[grp]

    gidx_lo = np.zeros((NCORES, TLO * 128), np.int16)
    gidx_hi = np.zeros((NCORES, THI * 128), np.int16)
    dloc_lo = np.full((NCORES, 128, TLO), -1.0, np.float32)
    dloc_hi = np.full((NCORES, 128, THI), -1.0, np.float32)
    for cc in range(NCORES):
        for h, (gidx, dloc, OFF, base) in enumerate(
            [(gidx_lo, dloc_lo, LOFF, 0), (gidx_hi, dloc_hi, HOFF, LO)]
        ):
            m = (cd_s == cc) & (hi_s == h)
            slot = OFF[td_s[m]] + rank[m] // 128
            part = rank[m] & 127
            gidx[cc, slot * 128 + part] = (psrc_s[m] - base).astype(np.int16)
            dloc[cc, part, slot] = dl_s[m].astype(np.float32)

    # ---- dis tables from host-side degrees (replaces on-device degree pass)
    deg = np.bincount(src, minlength=N).astype(np.float64)
    dis_node = np.where(deg > 0, 1.0 / np.sqrt(np.maximum(deg, 1.0)), 0.0)
    dis_t = np.zeros((NCORES, 128, NT), np.float32)
    for cc in range(NCORES):
        dn = np.zeros(NPAD, np.float64)
        dn[:NPC] = dis_node[cc * NPC : (cc + 1) * NPC]
        dis_t[cc] = dn.reshape(NT, 128).T.astype(np.float32)

    def wrap(a):  # int16 [M*128] -> [128, M*8], idx j at [j%16, j//16], x8 replicated
        return np.tile(a.reshape(-1, 16).T, (8, 1)).copy()

    return dict(
        Klo=Klo, Khi=Khi, LOFF=LOFF, HOFF=HOFF, TLO=TLO, THI=THI,
        gidx_lo=[wrap(gidx_lo[cc]) for cc in range(NCORES)],
        gidx_hi=[wrap(gidx_hi[cc]) for cc in range(NCORES)],
        dloc_lo=dloc_lo, dloc_hi=dloc_hi,
        dis=dis_t, negdis=-dis_t, n2dis2=(-2.0 * dis_t * dis_t).astype(np.float32),
    )


# ---------------------------------------------------------------- device build
def _build(c, pp, Fins, use_bias):
    import concourse.bacc as bacc
    import concourse.tile as tile
    from concourse import mybir

    f32, i16 = mybir.dt.float32, mybir.dt.int16
    bf16 = mybir.dt.bfloat16
    AOT = mybir.AluOpType
    NT, NPAD, NG, LO, FW = c["NT"], c["NPAD"], c["NG"], c["LO"], c["FW"]
    NCORES, F0, F2 = c["NCORES"], c["F0"], c["F2"]
    TLO, THI = pp["TLO"], pp["THI"]
    Klo, Khi = pp["Klo"], pp["Khi"]
    LOFF, HOFF = pp["LOFF"], pp["HOFF"]
    KMAX = int(max(Klo.max(), Khi.max()))
    NQ = 4  # SWDGE queues to rotate gather calls over
    GRP = 4
    groups = [list(range(g * GRP, min((g + 1) * GRP, NT))) for g in range(-(-NT // GRP))]

    nc = bacc.Bacc(num_devices=NCORES, num_swdge_queues=NQ)

    xin = nc.dram_tensor("x", [NPAD, F0], f32, kind="ExternalInput")
    gl_d = nc.dram_tensor("gidx_lo", [128, TLO * 8], i16, kind="ExternalInput")
    gh_d = nc.dram_tensor("gidx_hi", [128, THI * 8], i16, kind="ExternalInput")
    dl_d = nc.dram_tensor("dloc_lo", [128, TLO], f32, kind="ExternalInput")
    dh_d = nc.dram_tensor("dloc_hi", [128, THI], f32, kind="ExternalInput")
    dis_d = nc.dram_tensor("dis", [128, NT], f32, kind="ExternalInput")
    ndis_d = nc.dram_tensor("negdis", [128, NT], f32, kind="ExternalInput")
    n2d2_d = nc.dram_tensor("n2dis2", [128, NT], f32, kind="ExternalInput")
    iota_d = nc.dram_tensor("iota", [128, 128], f32, kind="ExternalInput")
    id_d = nc.dram_tensor("ident", [128, 128], f32, kind="ExternalInput")
    W_d = {}
    for l in range(3):
        for nm in ("wa", "wb", "wc"):
            W_d[nm, l] = nc.dram_tensor(f"{nm}{l}", [Fins[l], FW], f32, kind="ExternalInput")
        if use_bias[l]:
            W_d["br", l] = nc.dram_tensor(f"br{l}", [128, FW], f32, kind="ExternalInput")
    y_d = nc.dram_tensor("y", [NPAD, F2], f32, kind="ExternalOutput")

    ag_in = [nc.dram_tensor(f"agin{i}", [NPAD, FW], f32) for i in range(6)]
    ag_out = [nc.dram_tensor(f"agout{i}", [NG, FW], f32, addr_space="Shared") for i in range(6)]
    oh_lo_d = nc.dram_tensor("ohlo", [128, TLO * 128], bf16)
    oh_hi_d = nc.dram_tensor("ohhi", [128, THI * 128], bf16)

    xv = xin.rearrange("(t p) f -> p t f", p=128)
    yv = y_d.rearrange("(t p) f -> p t f", p=128)

    with tile.TileContext(nc) as tc, ExitStack() as ctx:
        cst = ctx.enter_context(tc.tile_pool(name="cst", bufs=1))
        big = ctx.enter_context(tc.tile_pool(name="big", bufs=1))
        gp = ctx.enter_context(tc.tile_pool(name="gp", bufs=2))
        ohp = ctx.enter_context(tc.tile_pool(name="ohp", bufs=2))
        smp = ctx.enter_context(tc.tile_pool(name="smp", bufs=6))
        slb = ctx.enter_context(tc.tile_pool(name="slb", bufs=2))
        psA = ctx.enter_context(tc.tile_pool(name="psA", bufs=2, space="PSUM"))
        psT = ctx.enter_context(tc.tile_pool(name="psT", bufs=2, space="PSUM"))
        psD = ctx.enter_context(tc.tile_pool(name="psD", bufs=2, space="PSUM"))

        # constants
        iota = cst.tile([128, 128], f32)
        nc.sync.dma_start(iota[:], iota_d[:])
        ident = cst.tile([128, 128], f32)
        nc.sync.dma_start(ident[:], id_d[:])
        gl = cst.tile([128, TLO * 8], i16)
        nc.sync.dma_start(gl[:], gl_d[:])
        gh = cst.tile([128, THI * 8], i16)
        nc.sync.dma_start(gh[:], gh_d[:])
        dlo = cst.tile([128, TLO], f32)
        nc.sync.dma_start(dlo[:], dl_d[:])
        dhi = cst.tile([128, THI], f32)
        nc.sync.dma_start(dhi[:], dh_d[:])
        dis = cst.tile([128, NT], f32)
        nc.sync.dma_start(dis[:], dis_d[:])
        negdis = cst.tile([128, NT], f32)
        nc.sync.dma_start(negdis[:], ndis_d[:])
        n2dis2 = cst.tile([128, NT], f32)
        nc.sync.dma_start(n2dis2[:], n2d2_d[:])
        Wt = {}
        for k, d in W_d.items():
            Wt[k] = cst.tile([128, FW], f32, name=f"w_{k[0]}_{k[1]}", tag=f"w_{k[0]}_{k[1]}")
            nc.sync.dma_start(Wt[k][: d.shape[0], :], d[:])

        # ---------------- lap helper
        qctr = [0]

        def lap(agi, epi):
            """Gather rows of ag_out[agi] per edge, scatter-add per dst tile,
            call epi(t, acc_psum) with the [128,FW] PSUM partial sums.
            agi==0 builds the one-hot slabs on DVE and stores them to DRAM;
            agi>0 streams them back instead."""
            build = agi == 0
            src_lo = ag_out[agi][0:LO, :]
            src_hi = ag_out[agi][LO:NG, :]
            for tl in groups:
                a_lo, b_lo = int(LOFF[tl[0]]), int(LOFF[tl[-1] + 1])
                a_hi, b_hi = int(HOFF[tl[0]]), int(HOFF[tl[-1] + 1])
                nlo, nhi = b_lo - a_lo, b_hi - a_hi
                CAP = 8  # max 128-chunks (1024 idx) per dma_gather call
                glo = gp.tile([128, nlo, FW], f32, tag="glo")
                for o in range(0, nlo, CAP):
                    n = min(CAP, nlo - o)
                    nc.gpsimd.dma_gather(
                        glo[:, o : o + n, :], src_lo,
                        gl[:, (a_lo + o) * 8 : (a_lo + o + n) * 8],
                        num_idxs=n * 128, num_idxs_reg=n * 128, elem_size=FW,
                        queue_num=qctr[0] % NQ,
                    )
                    qctr[0] += 1
                ghi_t = gp.tile([128, nhi, FW], f32, tag="ghi")
                for o in range(0, nhi, CAP):
                    n = min(CAP, nhi - o)
                    nc.gpsimd.dma_gather(
                        ghi_t[:, o : o + n, :], src_hi,
                        gh[:, (a_hi + o) * 8 : (a_hi + o + n) * 8],
                        num_idxs=n * 128, num_idxs_reg=n * 128, elem_size=FW,
                        queue_num=qctr[0] % NQ,
                    )
                    qctr[0] += 1
                glo_b = gp.tile([128, nlo, FW], bf16, tag="glob")
                nc.scalar.copy(glo_b[:], glo[:])
                ghi_b = gp.tile([128, nhi, FW], bf16, tag="ghib")
                nc.scalar.copy(ghi_b[:], ghi_t[:])
                for t in tl:
                    klo, khi = int(Klo[t]), int(Khi[t])
                    slab_lo = ohp.tile([128, KMAX * 128], bf16, tag="slab_lo")
                    slab_hi = ohp.tile([128, KMAX * 128], bf16, tag="slab_hi")
                    if build:
                        for k in range(klo):
                            s = int(LOFF[t]) + k
                            nc.vector.tensor_scalar(
                                slab_lo[:, k * 128 : (k + 1) * 128],
                                iota[:], dlo[:, s : s + 1], None, AOT.is_equal,
                            )
                        for k in range(khi):
                            s = int(HOFF[t]) + k
                            nc.vector.tensor_scalar(
                                slab_hi[:, k * 128 : (k + 1) * 128],
                                iota[:], dhi[:, s : s + 1], None, AOT.is_equal,
                            )
                        nc.sync.dma_start(
                            oh_lo_d[:, int(LOFF[t]) * 128 : (int(LOFF[t]) + klo) * 128],
                            slab_lo[:, : klo * 128],
                        )
                        nc.sync.dma_start(
                            oh_hi_d[:, int(HOFF[t]) * 128 : (int(HOFF[t]) + khi) * 128],
                            slab_hi[:, : khi * 128],
                        )
                    else:
                        nc.sync.dma_start(
                            slab_lo[:, : klo * 128],
                            oh_lo_d[:, int(LOFF[t]) * 128 : (int(LOFF[t]) + klo) * 128],
                        )
                        nc.sync.dma_start(
                            slab_hi[:, : khi * 128],
                            oh_hi_d[:, int(HOFF[t]) * 128 : (int(HOFF[t]) + khi) * 128],
                        )
                    acc = psA.tile([128, FW], f32, tag="acc")
                    ntot = klo + khi
                    i = 0
                    for k in range(klo):
                        s = int(LOFF[t]) + k
                        nc.tensor.matmul(
                            acc[:], slab_lo[:, k * 128 : (k + 1) * 128],
                            glo_b[:, s - a_lo, :],
                            start=(i == 0), stop=(i == ntot - 1),
                        )
                        i += 1
                    for k in range(khi):
                        s = int(HOFF[t]) + k
                        nc.tensor.matmul(
                            acc[:], slab_hi[:, k * 128 : (k + 1) * 128],
                            ghi_b[:, s - a_hi, :],
                            start=(i == 0), stop=(i == ntot - 1),
                        )
                        i += 1
                    epi(t, acc)

        # ---------------- layers
        h_prev = None
        for l in range(3):
            Fin = Fins[l]
            As = big.tile([128, NT, FW], f32, tag="As")
            Cs1 = big.tile([128, NT, FW], f32, tag="Cs1")
            Oa = big.tile([128, NT, FW], f32, tag="Oa")
            for tl in groups:
                w = len(tl) * 128
                hT = slb.tile([128, GRP * 128], f32, tag="hT")
                hsT = slb.tile([128, GRP * 128], f32, tag="hsT")
                for u, t in enumerate(tl):
                    if l == 0:
                        ht = smp.tile([128, F0], f32, tag="xt")
                        nc.sync.dma_start(ht[:], xv[:, t, :])
                        ht_ap = ht[:]
                    else:
                        ht_ap = h_prev[:, t, :]
                    ps = psT.tile([128, 128], f32, tag="pt")
                    nc.tensor.transpose(ps[:Fin, :], ht_ap, ident[:])
                    nc.scalar.copy(hT[:Fin, u * 128 : (u + 1) * 128], ps[:Fin, :])
                    hs = smp.tile([128, Fin], f32, tag="hs")
                    nc.vector.tensor_scalar_mul(hs[:], ht_ap, dis[:, t : t + 1])
                    ps2 = psT.tile([128, 128], f32, tag="pt")
                    nc.tensor.transpose(ps2[:Fin, :], hs[:], ident[:])
                    nc.scalar.copy(hsT[:Fin, u * 128 : (u + 1) * 128], ps2[:Fin, :])
                for dstbuf, wkey, srcT in (
                    (As, ("wc", l), hsT),
                    (Cs1, ("wb", l), hsT),
                    (Oa, ("wa", l), hT),
                ):
                    pd = psD.tile([64, GRP * 128], f32, tag="pd")
                    nc.tensor.matmul(pd[:, :w], Wt[wkey][:Fin, :], srcT[:Fin, :w])
                    pT = slb.tile([64, GRP * 128], f32, tag="pT")
                    nc.scalar.copy(pT[:, :w], pd[:, :w])
                    for u, t in enumerate(tl):
                        pb = psT.tile([128, 128], f32, tag="pt")
                        nc.tensor.transpose(
                            pb[:, :FW], pT[:FW, u * 128 : (u + 1) * 128], ident[:FW, :FW]
                        )
                        nc.scalar.copy(dstbuf[:, t, :], pb[:, :FW])

            agA = 2 * l
            nc.sync.dma_start(ag_in[agA].rearrange("(t p) f -> p t f", p=128), As[:])
            nc.gpsimd.collective_compute(
                "AllGather", mybir.AluOpType.bypass,
                replica_groups=[list(range(NCORES))],
                ins=[ag_in[agA][:, :]], outs=[ag_out[agA][:, :]],
            )

            Cs = big.tile([128, NT, FW], f32, tag="Cs")

            def epi1(t, acc):
                tmp = smp.tile([128, FW], f32, tag="t1")
                nc.vector.tensor_scalar_mul(tmp[:], acc[:], n2dis2[:, t : t + 1])
                nc.vector.tensor_add(Cs[:, t, :], Cs1[:, t, :], tmp[:])

            lap(agA, epi1)

            agC = 2 * l + 1
            nc.sync.dma_start(ag_in[agC].rearrange("(t p) f -> p t f", p=128), Cs[:])
            nc.gpsimd.collective_compute(
                "AllGather", mybir.AluOpType.bypass,
                replica_groups=[list(range(NCORES))],
                ins=[ag_in[agC][:, :]], outs=[ag_out[agC][:, :]],
            )

            hn = big.tile([128, NT, FW], f32, tag=f"h{l % 2}")

            def epi2(t, acc):
                tmp = smp.tile([128, FW], f32, tag="t1")
                nc.vector.tensor_scalar_mul(tmp[:], acc[:], negdis[:, t : t + 1])
                if use_bias[l]:
                    tmp2 = smp.tile([128, FW], f32, tag="t2")
                    nc.vector.tensor_add(tmp2[:], tmp[:], Oa[:, t, :])
                    pre = smp.tile([128, FW], f32, tag="t3")
                    nc.vector.tensor_add(pre[:], tmp2[:], Wt["br", l][:, :])
                else:
                    pre = smp.tile([128, FW], f32, tag="t2")
                    nc.vector.tensor_add(pre[:], tmp[:], Oa[:, t, :])
                if l < 2:
                    nc.vector.tensor_scalar_max(hn[:, t, :], pre[:], 0.0)
                else:
                    nc.vector.tensor_copy(hn[:, t, :], pre[:])

            lap(agC, epi2)
            h_prev = hn

        nc.sync.dma_start(yv[:], h_prev[:, :, :F2])

    nc.compile()
    return nc


# ---------------------------------------------------------------- entry
def _run(x, edge_index, Ws, bs, cfg=None, trace=False):
    from concourse.bass_utils import run_bass_kernel_spmd

    c = _derive(cfg or _REAL)
    N, NCORES, NPC, NPAD = c["N"], c["NCORES"], c["NPC"], c["NPAD"]
    F0, F2, FW = c["F0"], c["F2"], c["FW"]

    x = np.ascontiguousarray(np.asarray(x, dtype=np.float32))
    pp = _prep(edge_index, c)

    Fins = [F0, c["F1"], c["F1"]]
    use_bias = [bool(np.any(b)) for b in bs]
    nc = _build(c, pp, Fins, use_bias)

    iota = np.tile(np.arange(128, dtype=np.float32), (128, 1))
    ident = np.eye(128, dtype=np.float32)

    def padW(w, fin):
        out = np.zeros((fin, FW), np.float32)
        out[: w.shape[0], : w.shape[1]] = w
        return out

    base = {"iota": iota, "ident": ident}
    for l in range(3):
        W = np.asarray(Ws[l], dtype=np.float32)
        base[f"wa{l}"] = padW(W[0] - W[2], Fins[l])
        base[f"wb{l}"] = padW(W[1], Fins[l])
        base[f"wc{l}"] = padW(W[2], Fins[l])
        if use_bias[l]:
            br = np.zeros((128, FW), np.float32)
            br[:, : bs[l].shape[0]] = np.asarray(bs[l], np.float32)
            base[f"br{l}"] = br

    in_maps = []
    for cc in range(NCORES):
        xl = np.zeros((NPAD, F0), np.float32)
        xl[:NPC] = x[cc * NPC : (cc + 1) * NPC]
        in_maps.append(
            dict(
                base,
                x=xl,
                gidx_lo=pp["gidx_lo"][cc],
                gidx_hi=pp["gidx_hi"][cc],
                dloc_lo=np.ascontiguousarray(pp["dloc_lo"][cc]),
                dloc_hi=np.ascontiguousarray(pp["dloc_hi"][cc]),
                dis=np.ascontiguousarray(pp["dis"][cc]),
                negdis=np.ascontiguousarray(pp["negdis"][cc]),
                n2dis2=np.ascontiguousarray(pp["n2dis2"][cc]),
            )
        )

    res = run_bass_kernel_spmd(nc, in_maps, core_ids=list(range(NCORES)), trace=trace)
    out = np.concatenate([res.results[cc]["y"][:NPC] for cc in range(NCORES)], axis=0)
    return out[:, :F2], res


def kernel(x, edge_index, W1, b1, Wm, bm, W2, b2):
    out, _ = _run(
        np.asarray(x), np.asarray(edge_index),
        [np.asarray(W1), np.asarray(Wm), np.asarray(W2)],
        [np.asarray(b1), np.asarray(bm), np.asarray(b2)],
    )
    return out


# revision 12
# speedup vs baseline: 2.1936x; 1.1619x over previous
"""ChebNet (K=3, 3 layers) GNN on 8 Trainium2 NeuronCores.

Math: per layer, out = h@(W0-W2) + L(h@W1 + 2*L(h@W2)) + b, where
L(v) = -dis * S(dis * v), S = unweighted scatter-add over edges, and
dis = rsqrt(clamp(outdeg,1)) masked by outdeg>0.  The per-edge weight
w = dis[src]*dis[dst] folds into two dense per-node row scalings.

Sharding: nodes split into 8 contiguous ranges (one per core, padded to
49*128 rows).  Each core owns the edges whose dst falls in its range and
computes output rows for its range only.  Before each graph op (lap) the
dis-scaled operand is AllGathered so every core can gather arbitrary src
rows with dma_gather.  dma_gather indices are int16, so the 50176-row
gathered tensor is addressed through two windows (rows [0,32768) and
[32768,50176)); each dst-tile's edges are split lo/hi by src window and
the two partial scatter sums merge for free in PSUM accumulation.

Scatter-add is done on the tensor engine: for each 128-edge chunk, a
[128e x 128dst] one-hot matrix is matmul'd against the gathered
[128e x 64f] rows, accumulating [128dst x 64f] in PSUM.  The one-hot
matrices depend only on the graph: they are built on the vector engine
once (first lap), stored to DRAM, and streamed back by DMA for the
remaining 5 laps.  dis tables come precomputed from the host.  Gather
calls rotate across 4 SWDGE queues (distinct Q7 core pairs + rings).
"""

import sys

sys.path.insert(0, "/opt/trn_rl_repo")

import numpy as np
from contextlib import ExitStack

_REAL = dict(N=50000, E=800000, NCORES=8, LO=32768, F0=128, F1=64, F2=16)


# ---------------------------------------------------------------- host prep
def _derive(cfg):
    c = dict(cfg)
    c["NPC"] = c["N"] // c["NCORES"]
    c["NT"] = -(-c["NPC"] // 128)
    c["NPAD"] = c["NT"] * 128
    c["NG"] = c["NCORES"] * c["NPAD"]
    c["FW"] = 64  # lap working width (256B gather elements)
    assert c["LO"] <= 32768 and c["NG"] - c["LO"] <= 32768
    assert c["N"] % c["NCORES"] == 0
    return c


def _prep(edge_index, c):
    N, E, NCORES, LO = c["N"], c["E"], c["NCORES"], c["LO"]
    NPC, NT, NPAD = c["NPC"], c["NT"], c["NPAD"]

    src = np.asarray(edge_index[0], dtype=np.int64)
    dst = np.asarray(edge_index[1], dtype=np.int64)
    assert src.shape == (E,) and dst.shape == (E,)
    psrc = (src // NPC) * NPAD + (src % NPC)  # padded global row of src

    cd = dst // NPC
    ld = dst - cd * NPC
    td = ld >> 7
    dl = ld & 127
    hi = (psrc >= LO).astype(np.int64)

    # ---- lap tables: edges grouped by (core, dst-tile, window), src-sorted
    counts = np.zeros((NCORES, NT, 2), np.int64)
    np.add.at(counts, (cd, td, hi), 1)
    Klo = np.maximum(1, -(-counts[:, :, 0].max(0) // 128))
    Khi = np.maximum(1, -(-counts[:, :, 1].max(0) // 128))
    LOFF = np.concatenate([[0], np.cumsum(Klo)]).astype(np.int64)
    HOFF = np.concatenate([[0], np.cumsum(Khi)]).astype(np.int64)
    TLO, THI = int(LOFF[-1]), int(HOFF[-1])

    order = np.lexsort((psrc, hi, td, cd))
    cd_s, td_s, hi_s = cd[order], td[order], hi[order]
    dl_s, psrc_s = dl[order], psrc[order]
    grp = (cd_s * NT + td_s) * 2 + hi_s
    gc = np.bincount(grp, minlength=NCORES * NT * 2)
    gstart = np.concatenate([[0], np.cumsum(gc)])[:-1]
    rank = np.arange(E) - gstart[grp]

    gidx_lo = np.zeros((NCORES, TLO * 128), np.int16)
    gidx_hi = np.zeros((NCORES, THI * 128), np.int16)
    dloc_lo = np.full((NCORES, 128, TLO), -1.0, np.float32)
    dloc_hi = np.full((NCORES, 128, THI), -1.0, np.float32)
    for cc in range(NCORES):
        for h, (gidx, dloc, OFF, base) in enumerate(
            [(gidx_lo, dloc_lo, LOFF, 0), (gidx_hi, dloc_hi, HOFF, LO)]
        ):
            m = (cd_s == cc) & (hi_s == h)
            slot = OFF[td_s[m]] + rank[m] // 128
            part = rank[m] & 127
            gidx[cc, slot * 128 + part] = (psrc_s[m] - base).astype(np.int16)
            dloc[cc, part, slot] = dl_s[m].astype(np.float32)

    # ---- dis tables from host-side degrees (replaces on-device degree pass)
    deg = np.bincount(src, minlength=N).astype(np.float64)
    dis_node = np.where(deg > 0, 1.0 / np.sqrt(np.maximum(deg, 1.0)), 0.0)
    dis_t = np.zeros((NCORES, 128, NT), np.float32)
    for cc in range(NCORES):
        dn = np.zeros(NPAD, np.float64)
        dn[:NPC] = dis_node[cc * NPC : (cc + 1) * NPC]
        dis_t[cc] = dn.reshape(NT, 128).T.astype(np.float32)

    def wrap(a):  # int16 [M*128] -> [128, M*8], idx j at [j%16, j//16], x8 replicated
        return np.tile(a.reshape(-1, 16).T, (8, 1)).copy()

    return dict(
        Klo=Klo, Khi=Khi, LOFF=LOFF, HOFF=HOFF, TLO=TLO, THI=THI,
        gidx_lo=[wrap(gidx_lo[cc]) for cc in range(NCORES)],
        gidx_hi=[wrap(gidx_hi[cc]) for cc in range(NCORES)],
        dloc_lo=dloc_lo, dloc_hi=dloc_hi,
        dis=dis_t, negdis=-dis_t, n2dis2=(-2.0 * dis_t * dis_t).astype(np.float32),
    )


# ---------------------------------------------------------------- device build
def _build(c, pp, Fins, use_bias):
    import concourse.bacc as bacc
    import concourse.tile as tile
    from concourse import mybir

    f32, i16 = mybir.dt.float32, mybir.dt.int16
    bf16 = mybir.dt.bfloat16
    AOT = mybir.AluOpType
    NT, NPAD, NG, LO, FW = c["NT"], c["NPAD"], c["NG"], c["LO"], c["FW"]
    NCORES, F0, F2 = c["NCORES"], c["F0"], c["F2"]
    TLO, THI = pp["TLO"], pp["THI"]
    Klo, Khi = pp["Klo"], pp["Khi"]
    LOFF, HOFF = pp["LOFF"], pp["HOFF"]
    KMAX = int(max(Klo.max(), Khi.max()))
    NQ = 4  # SWDGE queues to rotate gather calls over
    GRP = 4
    groups = [list(range(g * GRP, min((g + 1) * GRP, NT))) for g in range(-(-NT // GRP))]

    nc = bacc.Bacc(num_devices=NCORES, num_swdge_queues=NQ)

    xin = nc.dram_tensor("x", [NPAD, F0], f32, kind="ExternalInput")
    gl_d = nc.dram_tensor("gidx_lo", [128, TLO * 8], i16, kind="ExternalInput")
    gh_d = nc.dram_tensor("gidx_hi", [128, THI * 8], i16, kind="ExternalInput")
    dl_d = nc.dram_tensor("dloc_lo", [128, TLO], f32, kind="ExternalInput")
    dh_d = nc.dram_tensor("dloc_hi", [128, THI], f32, kind="ExternalInput")
    dis_d = nc.dram_tensor("dis", [128, NT], f32, kind="ExternalInput")
    ndis_d = nc.dram_tensor("negdis", [128, NT], f32, kind="ExternalInput")
    n2d2_d = nc.dram_tensor("n2dis2", [128, NT], f32, kind="ExternalInput")
    iota_d = nc.dram_tensor("iota", [128, 128], f32, kind="ExternalInput")
    id_d = nc.dram_tensor("ident", [128, 128], f32, kind="ExternalInput")
    W_d = {}
    for l in range(3):
        for nm in ("wa", "wb", "wc"):
            W_d[nm, l] = nc.dram_tensor(f"{nm}{l}", [Fins[l], FW], f32, kind="ExternalInput")
        if use_bias[l]:
            W_d["br", l] = nc.dram_tensor(f"br{l}", [128, FW], f32, kind="ExternalInput")
    y_d = nc.dram_tensor("y", [NPAD, F2], f32, kind="ExternalOutput")

    ag_in = [nc.dram_tensor(f"agin{i}", [NPAD, FW], f32) for i in range(6)]
    ag_out = [nc.dram_tensor(f"agout{i}", [NG, FW], f32, addr_space="Shared") for i in range(6)]
    oh_lo_d = nc.dram_tensor("ohlo", [128, TLO * 128], bf16)
    oh_hi_d = nc.dram_tensor("ohhi", [128, THI * 128], bf16)

    xv = xin.rearrange("(t p) f -> p t f", p=128)
    yv = y_d.rearrange("(t p) f -> p t f", p=128)

    with tile.TileContext(nc) as tc, ExitStack() as ctx:
        cst = ctx.enter_context(tc.tile_pool(name="cst", bufs=1))
        big = ctx.enter_context(tc.tile_pool(name="big", bufs=1))
        gp = ctx.enter_context(tc.tile_pool(name="gp", bufs=2))
        ohp = ctx.enter_context(tc.tile_pool(name="ohp", bufs=2))
        smp = ctx.enter_context(tc.tile_pool(name="smp", bufs=6))
        slb = ctx.enter_context(tc.tile_pool(name="slb", bufs=2))
        psA = ctx.enter_context(tc.tile_pool(name="psA", bufs=2, space="PSUM"))
        psT = ctx.enter_context(tc.tile_pool(name="psT", bufs=2, space="PSUM"))
        psD = ctx.enter_context(tc.tile_pool(name="psD", bufs=2, space="PSUM"))

        # constants
        iota = cst.tile([128, 128], f32)
        nc.sync.dma_start(iota[:], iota_d[:])
        ident = cst.tile([128, 128], f32)
        nc.sync.dma_start(ident[:], id_d[:])
        gl = cst.tile([128, TLO * 8], i16)
        nc.sync.dma_start(gl[:], gl_d[:])
        gh = cst.tile([128, THI * 8], i16)
        nc.sync.dma_start(gh[:], gh_d[:])
        dlo = cst.tile([128, TLO], f32)
        nc.sync.dma_start(dlo[:], dl_d[:])
        dhi = cst.tile([128, THI], f32)
        nc.sync.dma_start(dhi[:], dh_d[:])
        dis = cst.tile([128, NT], f32)
        nc.sync.dma_start(dis[:], dis_d[:])
        negdis = cst.tile([128, NT], f32)
        nc.sync.dma_start(negdis[:], ndis_d[:])
        n2dis2 = cst.tile([128, NT], f32)
        nc.sync.dma_start(n2dis2[:], n2d2_d[:])
        Wt = {}
        Wb = {}
        for k, d in W_d.items():
            Wt[k] = cst.tile([128, FW], f32, name=f"w_{k[0]}_{k[1]}", tag=f"w_{k[0]}_{k[1]}")
            nc.sync.dma_start(Wt[k][: d.shape[0], :], d[:])
            if k[0] != "br":
                Wb[k] = cst.tile([128, FW], bf16, name=f"wb_{k[0]}_{k[1]}", tag=f"wb_{k[0]}_{k[1]}")
                nc.scalar.copy(Wb[k][: d.shape[0], :], Wt[k][: d.shape[0], :])
        ident_b = cst.tile([128, 128], bf16)
        nc.scalar.copy(ident_b[:], ident[:])

        # ---------------- lap helper
        qctr = [0]

        def lap(agi, epi):
            """Gather rows of ag_out[agi] per edge, scatter-add per dst tile,
            call epi(t, acc_psum) with the [128,FW] PSUM partial sums.
            agi==0 builds the one-hot slabs on DVE and stores them to DRAM;
            agi>0 streams them back instead."""
            build = agi == 0
            src_lo = ag_out[agi][0:LO, :]
            src_hi = ag_out[agi][LO:NG, :]
            for tl in groups:
                a_lo, b_lo = int(LOFF[tl[0]]), int(LOFF[tl[-1] + 1])
                a_hi, b_hi = int(HOFF[tl[0]]), int(HOFF[tl[-1] + 1])
                nlo, nhi = b_lo - a_lo, b_hi - a_hi
                CAP = 8  # max 128-chunks (1024 idx) per dma_gather call
                glo = gp.tile([128, nlo, FW], f32, tag="glo")
                for o in range(0, nlo, CAP):
                    n = min(CAP, nlo - o)
                    nc.gpsimd.dma_gather(
                        glo[:, o : o + n, :], src_lo,
                        gl[:, (a_lo + o) * 8 : (a_lo + o + n) * 8],
                        num_idxs=n * 128, num_idxs_reg=n * 128, elem_size=FW,
                        queue_num=qctr[0] % NQ,
                    )
                    qctr[0] += 1
                ghi_t = gp.tile([128, nhi, FW], f32, tag="ghi")
                for o in range(0, nhi, CAP):
                    n = min(CAP, nhi - o)
                    nc.gpsimd.dma_gather(
                        ghi_t[:, o : o + n, :], src_hi,
                        gh[:, (a_hi + o) * 8 : (a_hi + o + n) * 8],
                        num_idxs=n * 128, num_idxs_reg=n * 128, elem_size=FW,
                        queue_num=qctr[0] % NQ,
                    )
                    qctr[0] += 1
                glo_b = gp.tile([128, nlo, FW], bf16, tag="glob")
                nc.scalar.copy(glo_b[:], glo[:])
                ghi_b = gp.tile([128, nhi, FW], bf16, tag="ghib")
                nc.scalar.copy(ghi_b[:], ghi_t[:])
                for t in tl:
                    klo, khi = int(Klo[t]), int(Khi[t])
                    slab_lo = ohp.tile([128, KMAX * 128], bf16, tag="slab_lo")
                    slab_hi = ohp.tile([128, KMAX * 128], bf16, tag="slab_hi")
                    if build:
                        for k in range(klo):
                            s = int(LOFF[t]) + k
                            nc.vector.tensor_scalar(
                                slab_lo[:, k * 128 : (k + 1) * 128],
                                iota[:], dlo[:, s : s + 1], None, AOT.is_equal,
                            )
                        for k in range(khi):
                            s = int(HOFF[t]) + k
                            nc.vector.tensor_scalar(
                                slab_hi[:, k * 128 : (k + 1) * 128],
                                iota[:], dhi[:, s : s + 1], None, AOT.is_equal,
                            )
                        nc.sync.dma_start(
                            oh_lo_d[:, int(LOFF[t]) * 128 : (int(LOFF[t]) + klo) * 128],
                            slab_lo[:, : klo * 128],
                        )
                        nc.sync.dma_start(
                            oh_hi_d[:, int(HOFF[t]) * 128 : (int(HOFF[t]) + khi) * 128],
                            slab_hi[:, : khi * 128],
                        )
                    else:
                        nc.sync.dma_start(
                            slab_lo[:, : klo * 128],
                            oh_lo_d[:, int(LOFF[t]) * 128 : (int(LOFF[t]) + klo) * 128],
                        )
                        nc.sync.dma_start(
                            slab_hi[:, : khi * 128],
                            oh_hi_d[:, int(HOFF[t]) * 128 : (int(HOFF[t]) + khi) * 128],
                        )
                    acc = psA.tile([128, FW], f32, tag="acc")
                    ntot = klo + khi
                    i = 0
                    for k in range(klo):
                        s = int(LOFF[t]) + k
                        nc.tensor.matmul(
                            acc[:], slab_lo[:, k * 128 : (k + 1) * 128],
                            glo_b[:, s - a_lo, :],
                            start=(i == 0), stop=(i == ntot - 1),
                        )
                        i += 1
                    for k in range(khi):
                        s = int(HOFF[t]) + k
                        nc.tensor.matmul(
                            acc[:], slab_hi[:, k * 128 : (k + 1) * 128],
                            ghi_b[:, s - a_hi, :],
                            start=(i == 0), stop=(i == ntot - 1),
                        )
                        i += 1
                    epi(t, acc)

        # ---------------- layers
        HALF = (NT // 2) * 128  # sub-AllGather split point (rows)

        def sub_ag(agi, buf):
            nc.sync.dma_start(ag_in[agi].rearrange("(t p) f -> p t f", p=128), buf[:])
            nc.gpsimd.collective_compute(
                "AllGather", mybir.AluOpType.bypass,
                replica_groups=[list(range(NCORES))],
                ins=[ag_in[agi][:, :]], outs=[ag_out[agi][:, :]],
            )

        h_prev = None
        for l in range(3):
            Fin = Fins[l]
            As = big.tile([128, NT, FW], f32, tag="As")
            Cs1 = big.tile([128, NT, FW], f32, tag="Cs1")
            Oa = big.tile([128, NT, FW], f32, tag="Oa")
            hT_all = big.tile([128, NT * 128], bf16, tag="hTa")
            hsT_all = big.tile([128, NT * 128], bf16, tag="hsTa")
            # pass 1: transposes + As (the AllGather input) only
            for tl in groups:
                w = len(tl) * 128
                c0 = tl[0] * 128
                for u, t in enumerate(tl):
                    if l == 0:
                        ht = smp.tile([128, F0], f32, tag="xt")
                        nc.sync.dma_start(ht[:], xv[:, t, :])
                        ht_b = smp.tile([128, F0], bf16, tag="xtb")
                        nc.scalar.copy(ht_b[:], ht[:])
                        ht_ap = ht_b[:]
                    else:
                        ht_ap = h_prev[:, t, :]
                    ps = psT.tile([128, 128], bf16, tag="pt")
                    nc.tensor.transpose(ps[:Fin, :], ht_ap, ident_b[:])
                    nc.scalar.copy(hT_all[:Fin, (t * 128) : (t + 1) * 128], ps[:Fin, :])
                    hs = smp.tile([128, Fin], bf16, tag="hs")
                    nc.vector.tensor_scalar_mul(hs[:], ht_ap, dis[:, t : t + 1])
                    ps2 = psT.tile([128, 128], bf16, tag="pt")
                    nc.tensor.transpose(ps2[:Fin, :], hs[:], ident_b[:])
                    nc.scalar.copy(hsT_all[:Fin, (t * 128) : (t + 1) * 128], ps2[:Fin, :])
                pd = psD.tile([64, GRP * 128], f32, tag="pd")
                nc.tensor.matmul(pd[:, :w], Wb["wc", l][:Fin, :], hsT_all[:Fin, c0 : c0 + w])
                pT = slb.tile([64, GRP * 128], bf16, tag="pT")
                nc.scalar.copy(pT[:, :w], pd[:, :w])
                for u, t in enumerate(tl):
                    pb = psT.tile([128, 128], bf16, tag="pt")
                    nc.tensor.transpose(
                        pb[:, :FW], pT[:FW, u * 128 : (u + 1) * 128], ident_b[:FW, :FW]
                    )
                    nc.scalar.copy(As[:, t, :], pb[:, :FW])

            agA = 2 * l
            sub_ag(agA, As)

            # pass 2: Cs1 and Oa — overlaps the first lap's gathers
            for tl in groups:
                w = len(tl) * 128
                c0 = tl[0] * 128
                for dstbuf, wkey, srcT in (
                    (Cs1, ("wb", l), hsT_all),
                    (Oa, ("wa", l), hT_all),
                ):
                    pd = psD.tile([64, GRP * 128], f32, tag="pd")
                    nc.tensor.matmul(pd[:, :w], Wb[wkey][:Fin, :], srcT[:Fin, c0 : c0 + w])
                    pT = slb.tile([64, GRP * 128], bf16, tag="pT")
                    nc.scalar.copy(pT[:, :w], pd[:, :w])
                    for u, t in enumerate(tl):
                        pb = psT.tile([128, 128], bf16, tag="pt")
                        nc.tensor.transpose(
                            pb[:, :FW], pT[:FW, u * 128 : (u + 1) * 128], ident_b[:FW, :FW]
                        )
                        nc.scalar.copy(dstbuf[:, t, :], pb[:, :FW])

            Cs = big.tile([128, NT, FW], f32, tag="Cs")

            def epi1(t, acc):
                tmp = smp.tile([128, FW], f32, tag="t1")
                nc.vector.tensor_scalar_mul(tmp[:], acc[:], n2dis2[:, t : t + 1])
                nc.vector.tensor_add(Cs[:, t, :], Cs1[:, t, :], tmp[:])

            lap(agA, epi1)

            agC = 2 * l + 1
            sub_ag(agC, Cs)

            hn = big.tile([128, NT, FW], bf16 if l < 2 else f32, tag=f"h{l}")

            def epi2(t, acc):
                tmp = smp.tile([128, FW], f32, tag="t1")
                nc.vector.tensor_scalar_mul(tmp[:], acc[:], negdis[:, t : t + 1])
                if use_bias[l]:
                    tmp2 = smp.tile([128, FW], f32, tag="t2")
                    nc.vector.tensor_add(tmp2[:], tmp[:], Oa[:, t, :])
                    pre = smp.tile([128, FW], f32, tag="t3")
                    nc.vector.tensor_add(pre[:], tmp2[:], Wt["br", l][:, :])
                else:
                    pre = smp.tile([128, FW], f32, tag="t2")
                    nc.vector.tensor_add(pre[:], tmp[:], Oa[:, t, :])
                if l < 2:
                    nc.vector.tensor_scalar_max(hn[:, t, :], pre[:], 0.0)
                else:
                    nc.vector.tensor_copy(hn[:, t, :], pre[:])

            lap(agC, epi2)
            h_prev = hn

        nc.sync.dma_start(yv[:], h_prev[:, :, :F2])

    nc.compile()
    return nc


# ---------------------------------------------------------------- entry
def _run(x, edge_index, Ws, bs, cfg=None, trace=False):
    from concourse.bass_utils import run_bass_kernel_spmd

    c = _derive(cfg or _REAL)
    N, NCORES, NPC, NPAD = c["N"], c["NCORES"], c["NPC"], c["NPAD"]
    F0, F2, FW = c["F0"], c["F2"], c["FW"]

    x = np.ascontiguousarray(np.asarray(x, dtype=np.float32))
    pp = _prep(edge_index, c)

    Fins = [F0, c["F1"], c["F1"]]
    use_bias = [bool(np.any(b)) for b in bs]
    nc = _build(c, pp, Fins, use_bias)

    iota = np.tile(np.arange(128, dtype=np.float32), (128, 1))
    ident = np.eye(128, dtype=np.float32)

    def padW(w, fin):
        out = np.zeros((fin, FW), np.float32)
        out[: w.shape[0], : w.shape[1]] = w
        return out

    base = {"iota": iota, "ident": ident}
    for l in range(3):
        W = np.asarray(Ws[l], dtype=np.float32)
        base[f"wa{l}"] = padW(W[0] - W[2], Fins[l])
        base[f"wb{l}"] = padW(W[1], Fins[l])
        base[f"wc{l}"] = padW(W[2], Fins[l])
        if use_bias[l]:
            br = np.zeros((128, FW), np.float32)
            br[:, : bs[l].shape[0]] = np.asarray(bs[l], np.float32)
            base[f"br{l}"] = br

    in_maps = []
    for cc in range(NCORES):
        xl = np.zeros((NPAD, F0), np.float32)
        xl[:NPC] = x[cc * NPC : (cc + 1) * NPC]
        in_maps.append(
            dict(
                base,
                x=xl,
                gidx_lo=pp["gidx_lo"][cc],
                gidx_hi=pp["gidx_hi"][cc],
                dloc_lo=np.ascontiguousarray(pp["dloc_lo"][cc]),
                dloc_hi=np.ascontiguousarray(pp["dloc_hi"][cc]),
                dis=np.ascontiguousarray(pp["dis"][cc]),
                negdis=np.ascontiguousarray(pp["negdis"][cc]),
                n2dis2=np.ascontiguousarray(pp["n2dis2"][cc]),
            )
        )

    res = run_bass_kernel_spmd(nc, in_maps, core_ids=list(range(NCORES)), trace=trace)
    out = np.concatenate([res.results[cc]["y"][:NPC] for cc in range(NCORES)], axis=0)
    return out[:, :F2], res


def kernel(x, edge_index, W1, b1, Wm, bm, W2, b2):
    out, _ = _run(
        np.asarray(x), np.asarray(edge_index),
        [np.asarray(W1), np.asarray(Wm), np.asarray(W2)],
        [np.asarray(b1), np.asarray(bm), np.asarray(b2)],
    )
    return out
